# revision 21
# baseline (speedup 1.0000x reference)
"""Trainium2 Bass kernel: CrossAttention  (B=16, S=4096, D_IN=512, D=1024, H=16, HD=64).

reference math:
    x1e = x1@We1+be1; x2e = x2@We2+be2; x3e = x3@We2+be2
    q = x1e@Wq+bq; k = x2e@Wk+bk; v = x3e@Wv+bv     (per-head split, HD=64)
    attn = softmax(q.k/sqrt(HD)); av = attn.v; out = av@Wo+bo   -> [B, D]

Sharding: data-parallel over batch, 2 batches per core, 8 cores, no collectives.

Because the query is a SINGLE token per batch, q is tiny and is computed on
host; it is then folded into the K-side weights so the big K GEMM disappears:
    logits[h,s] = q'[h] . (x2[s]@W2k)[head h] = x2[s] @ wl[:,h],
        wl[:,h] = W2k[:, 64h:64h+64] @ q'[h],   q' = q/sqrt(HD)   (host, f64)
    (k bias shifts a head's logits by a constant -> softmax-invariant -> dropped)
The V GEMM is likewise eliminated by associativity:
    av = attn @ (x3@W2v) + bve = ((attn@x3) @ W2v) + bve
so the device only contracts attn (16 rows) against x3 once, then a tiny
[16,512]@[512,1024]. bve's contribution (attn rows sum to 1) is folded into
the output bias boe = bve@Wo + bo, added on host after the gather.

Device work per batch:
    lgts = wl^T @ x2^T  tiles        [H, S]     (x2 host-transposed, streamed)
    attn = exp(lgts - max), row sums kept; normalization folded into U
    attnT via PE transpose           [S, H]
    U    = attnT^T @ x3              [H, 512]   (x3 streamed in NATIVE layout)
    Un   = U * (1/sum)               [H, 512]
    O    = Un @ W2v                  [H, D];  av = diag-head blocks of O
    out  = blockdiag(av)^T @ Wo      [B_LOC, D]

Schedule: three DMA queues (gpsimd: x2, sync: x3, scalar: weights with wo
deferred to the tail), both batches' logits emitted back-to-back on the PE so
batch-1 streaming hides batch-0's softmax latency, per-tile partial maxes and
a split exp to shorten the softmax critical path.
"""

import os

import numpy as np

B, S, D_IN, D, H, HD = 16, 4096, 512, 1024, 16, 64
N_CORES = 8
B_LOC = B // N_CORES  # 2
KI = D_IN // 128      # 4 contraction chunks over D_IN
MO = D // 128         # 8 feature chunks over D
ST = S // 512         # 8 sequence tiles
SC = S // 128         # 32 sequence chunks
NT = D // 512         # 2 output-feature tiles


def _emit(nc, tc, ctx, mm_dt):
    import concourse.mybir as mybir

    dt = mybir.dt
    f32 = dt.float32
    AF = mybir.ActivationFunctionType
    AX = mybir.AxisListType
    ALU = mybir.AluOpType
    bf16 = mm_dt == dt.bfloat16
    t_dt = mm_dt if bf16 else f32

    # all activation/weight tensors host-packed tile-major: per-partition data
    # is contiguous (>=4KB lines) so every DMA runs at full HBM efficiency
    x2t = nc.declare_dram_parameter(
        "x2t", [B_LOC, ST // 2, 128, KI, 1024], mm_dt, isOutput=False
    )
    x3n = nc.declare_dram_parameter(
        "x3n", [B_LOC, ST // 2, 128, 8, D_IN], mm_dt, isOutput=False
    )
    wl = nc.declare_dram_parameter("wl", [128, B_LOC, KI, H], mm_dt, isOutput=False)
    w2v = nc.declare_dram_parameter("w2v", [128, KI, D], mm_dt, isOutput=False)
    wo = nc.declare_dram_parameter("wo", [128, MO, D], mm_dt, isOutput=False)
    eye_io = nc.declare_dram_parameter("eye_io", [H, H], mm_dt, isOutput=False)
    eye_f32 = nc.declare_dram_parameter("eye_f32", [H, H], f32, isOutput=False)
    out_p = nc.declare_dram_parameter("out", [B_LOC, D], f32, isOutput=True)

    wpool = ctx.enter_context(tc.tile_pool(name="weights", bufs=1))
    x2pool = ctx.enter_context(tc.tile_pool(name="x2in", bufs=5))
    x3pool = ctx.enter_context(tc.tile_pool(name="x3in", bufs=8))
    bpool = ctx.enter_context(tc.tile_pool(name="perbatch", bufs=2))
    spool = ctx.enter_context(tc.tile_pool(name="singles", bufs=1))
    ps = ctx.enter_context(tc.tile_pool(name="ps", bufs=1, space="PSUM"))

    # --- small weights first (scalar queue), wl before the eyes so the very
    # first logits matmul unblocks as early as possible; wo deferred ---
    wl_sb = spool.tile([128, B_LOC, KI, H], mm_dt, tag="wl")
    nc.scalar.dma_start(out=wl_sb, in_=wl[:, :, :, :])
    eye_io_sb = spool.tile([H, H], mm_dt, tag="eye_io")
    nc.scalar.dma_start(out=eye_io_sb, in_=eye_io[:, :])
    eye_f32_sb = spool.tile([H, H], f32, tag="eye_f32")
    nc.scalar.dma_start(out=eye_f32_sb, in_=eye_f32[:, :])
    w2v_sb = wpool.tile([128, KI, D], mm_dt, tag="w2v")
    nc.scalar.dma_start(out=w2v_sb, in_=w2v[:, :, :])

    # --- x2 on gpsimd, x3 on sync: both batches issued up front (a split
    # across more queues was tried and regressed: out-of-order tile arrival
    # stalls the in-order PE consumption) ---
    x2s_all = [[None] * (ST // 2) for _ in range(B_LOC)]
    x3s_all = [[None] * (ST // 2) for _ in range(B_LOC)]
    for b in range(B_LOC):
        for t in range(ST // 2):
            x2s = x2pool.tile([128, KI, 1024], mm_dt, tag="x2")
            nc.gpsimd.dma_start(out=x2s, in_=x2t[b, t])
            x2s_all[b][t] = x2s
    for b in range(B_LOC):
        for t in range(ST // 2):
            x3s = x3pool.tile([128, 8, D_IN], mm_dt, tag="x3")
            nc.sync.dma_start(out=x3s, in_=x3n[b, t])
            x3s_all[b][t] = x3s

    lg_all, at_all, rs_all = [], [], []

    # ---------------- logits for BOTH batches back-to-back on PE ----------------
    for b in range(B_LOC):
        lg = bpool.tile([H, S], f32, tag="lg")
        nmx8 = bpool.tile([H, ST], f32, tag="nmx8")
        for st in range(ST):
            t, half = st // 2, st % 2
            lp = ps.tile([H, 512], f32, tag="lg", bufs=2)
            for ki in range(KI):
                nc.tensor.matmul(
                    lp,
                    (wl_sb[:, b, ki, :]),
                    (x2s_all[b][t][:, ki, half * 512:(half + 1) * 512]),
                    start=(ki == 0),
                    stop=(ki == KI - 1),
                )
            nc.vector.tensor_copy(out=lg[:, st * 512:(st + 1) * 512], in_=lp)
            # partial max per tile straight from PSUM (short final reduce)
            nc.vector.tensor_reduce(
                out=nmx8[:, st:st + 1], in_=lp, axis=AX.X, op=ALU.max
            )
        nmx = bpool.tile([H, 1], f32, tag="nmx")
        nc.vector.tensor_reduce(
            out=nmx, in_=nmx8, axis=AX.X, op=ALU.max, negate=True
        )

        # softmax: exp in two halves to release transposes earlier
        if bf16:
            attn = bpool.tile([H, S], mm_dt, tag="attn")
            attn_eye = eye_io_sb
        else:
            attn = lg  # exp in place
            attn_eye = eye_f32_sb
        ssum2 = bpool.tile([H, 2], f32, tag="ssum2")
        for hlf in range(2):
            sl = slice(hlf * (S // 2), (hlf + 1) * (S // 2))
            nc.scalar.activation(
                out=attn[:, sl],
                in_=lg[:, sl],
                func=AF.Exp,
                bias=nmx,
                scale=1.0,
                accum_out=ssum2[:, hlf:hlf + 1],
            )
        lg_all.append((attn, attn_eye, ssum2))

    # sums/reciprocals AFTER both batches' logits so batch-1 PSUM evictions on
    # the in-order vector queue never wait behind batch-0's exp
    for b in range(B_LOC):
        ssum2 = lg_all[b][2]
        ssum = bpool.tile([H, 1], f32, tag="ssum")
        nc.vector.tensor_scalar_add(
            out=ssum, in0=ssum2[:, 0:1], scalar1=ssum2[:, 1:2]
        )
        rs = bpool.tile([H, 1], f32, tag="rs")
        nc.vector.reciprocal(out=rs, in_=ssum)
        rs_all.append(rs)

    # attn^T via PE transpose AFTER both batches' logits (keeps PE streaming
    # batch 1 logits while batch 0's exp runs)
    for b in range(B_LOC):
        attn, attn_eye, _ = lg_all[b]
        at = bpool.tile([128, SC, H], mm_dt, tag="at")
        for g in range(SC // 4):
            tp = ps.tile([128, 4, H], t_dt, tag="tp", bufs=2)
            for j in range(4):
                sc = g * 4 + j
                nc.tensor.transpose(
                    tp[:, j, :], attn[:, sc * 128:(sc + 1) * 128], attn_eye
                )
            nc.vector.tensor_copy(out=at[:, g * 4:(g + 1) * 4, :], in_=tp)
        at_all.append(at)

    # ---------------- per-batch: U, O, diag extraction ----------------
    avv = spool.tile([128, MO, B_LOC], mm_dt, tag="avv")  # av^T diag blocks
    for b in range(B_LOC):
        at, rs = at_all[b], rs_all[b]
        up = ps.tile([H, D_IN], f32, tag="u", bufs=2, name=f"up{b}")
        for t in range(ST // 2):
            for g in range(8):
                sc = t * 8 + g
                nc.tensor.matmul(
                    up,
                    (at[:, sc, :]),
                    (x3s_all[b][t][:, g, :]),
                    start=(sc == 0),
                    stop=(sc == SC - 1),
                )
        # normalize rows by 1/sum while evicting
        if bf16:
            un = bpool.tile([H, D_IN], mm_dt, tag="un")
            un_eye = eye_io_sb
        else:
            un = bpool.tile([H, D_IN], f32, tag="un")
            un_eye = eye_f32_sb
        nc.vector.tensor_scalar_mul(out=un, in0=up, scalar1=rs)

        # Un^T [D_IN, H] for the O GEMM
        unt = bpool.tile([128, KI, H], mm_dt, tag="unt")
        tpu = ps.tile([128, KI, H], t_dt, tag="tp", bufs=2)
        for c in range(KI):
            nc.tensor.transpose(
                tpu[:, c, :], un[:, c * 128:(c + 1) * 128], un_eye
            )
        nc.vector.tensor_copy(out=unt, in_=tpu)

        # O = Un @ W2v  [H, D]; av = diag-head blocks
        avs = bpool.tile([H, D], f32, tag="avs")
        for n in range(NT):
            op = ps.tile([H, 512], f32, tag="o", bufs=2)
            for ki in range(KI):
                nc.tensor.matmul(
                    op,
                    (unt[:, ki, :]),
                    (w2v_sb[:, ki, n * 512:(n + 1) * 512]),
                    start=(ki == 0),
                    stop=(ki == KI - 1),
                )
            nc.vector.tensor_copy(out=avs[:, n * 512:(n + 1) * 512], in_=op)

        # av^T, then extract the per-head diagonal blocks into avv
        avt = bpool.tile([128, MO, H], f32, tag="avt")
        for g in range(2):
            tpa = ps.tile([128, 4, H], f32, tag="tp", bufs=2)
            for j in range(4):
                mo = g * 4 + j
                nc.tensor.transpose(
                    tpa[:, j, :], avs[:, mo * 128:(mo + 1) * 128], eye_f32_sb
                )
            nc.vector.tensor_copy(out=avt[:, g * 4:(g + 1) * 4, :], in_=tpa)
        for mo in range(MO):
            nc.vector.tensor_copy(
                out=avv[0:64, mo, b:b + 1], in_=avt[0:64, mo, 2 * mo:2 * mo + 1]
            )
            nc.vector.tensor_copy(
                out=avv[64:128, mo, b:b + 1],
                in_=avt[64:128, mo, 2 * mo + 1:2 * mo + 2],
            )

    # wo arrives late on the scalar queue (emitted after both ACTIVATEs)
    wo_sb = wpool.tile([128, MO, D], mm_dt, tag="wo")
    nc.scalar.dma_start(out=wo_sb, in_=wo[:, :, :])

    # ---------------- out = avvec @ Wo  (both batches at once) ----------------
    out_sb = spool.tile([B_LOC, D], f32, tag="outsb")
    for n in range(NT):
        op = ps.tile([B_LOC, 512], f32, tag="lg", bufs=2)
        for mo in range(MO):
            nc.tensor.matmul(
                op,
                (avv[:, mo, :]),
                (wo_sb[:, mo, n * 512:(n + 1) * 512]),
                start=(mo == 0),
                stop=(mo == MO - 1),
            )
        nc.vector.tensor_copy(out=out_sb[:, n * 512:(n + 1) * 512], in_=op)
    nc.gpsimd.dma_start(out=out_p[:, :], in_=out_sb)


def build_program(mode=None):
    """mode: 'f32r' | 'bf16'. Returns a compiled Bass object."""
    from contextlib import ExitStack

    import concourse.mybir as mybir
    import concourse.tile as tile
    from concourse import bacc

    mode = mode or os.environ.get("BASSK_MODE", "bf16")
    mm_dt = {
        "f32": mybir.dt.float32,
        "f32r": mybir.dt.float32r,
        "bf16": mybir.dt.bfloat16,
    }[mode]

    nc = bacc.Bacc()
    with ExitStack() as ctx:
        tc = ctx.enter_context(tile.TileContext(nc))
        _emit(nc, tc, ctx, mm_dt)
    nc.compile()
    return nc


def prep_inputs(inputs, mode=None):
    """Host-side folding + per-core sharding. Returns (in_maps, boe)."""
    mode = mode or os.environ.get("BASSK_MODE", "bf16")
    g = {k: np.asarray(v, np.float64) for k, v in inputs.items()}
    W2k = g["We2"] @ g["Wk"]          # k bias dropped: softmax shift-invariant
    W2v = g["We2"] @ g["Wv"]
    q = (g["x1"][:, 0] @ g["We1"] + g["be1"]) @ g["Wq"] + g["bq"]   # [B, D]
    q /= np.sqrt(HD)
    # wl[b,:,h] = W2k[:, 64h:64h+64] @ q[b, 64h:64h+64]
    wl = np.einsum(
        "dhe,bhe->bdh", W2k.reshape(D_IN, H, HD), q.reshape(B, H, HD)
    )
    bve = g["be2"] @ g["Wv"] + g["bv"]
    boe = (bve @ g["Wo"] + g["bo"]).astype(np.float32)  # added on host at the end

    io_np = np.float32
    if mode == "bf16":
        import ml_dtypes

        io_np = ml_dtypes.bfloat16

    # tile-major packing: per-partition lines contiguous for full DMA rate
    x2 = np.asarray(inputs["x2"], np.float32).astype(io_np)
    x3 = np.asarray(inputs["x3"], np.float32).astype(io_np)
    # x2t[b, t, p, ki, s'] = x2[b, t*1024+s', ki*128+p]
    x2p = np.ascontiguousarray(
        x2.reshape(B, ST // 2, 1024, KI, 128).transpose(0, 1, 4, 3, 2)
    )
    # x3n[b, t, p, g, d] = x3[b, (t*8+g)*128+p, d]
    x3p = np.ascontiguousarray(
        x3.reshape(B, ST // 2, 8, 128, D_IN).transpose(0, 1, 3, 2, 4)
    )
    W2vp = np.ascontiguousarray(
        W2v.astype(np.float32).astype(io_np).reshape(KI, 128, D).transpose(1, 0, 2)
    )
    Wop = np.ascontiguousarray(
        np.asarray(inputs["Wo"], np.float32)
        .astype(io_np)
        .reshape(MO, 128, D)
        .transpose(1, 0, 2)
    )
    wlc = wl.astype(np.float32).astype(io_np)  # [B, D_IN, H]
    shared = {
        "w2v": W2vp,
        "wo": Wop,
        "eye_io": np.eye(H, dtype=io_np),
        "eye_f32": np.eye(H, dtype=np.float32),
    }
    in_maps = []
    for c in range(N_CORES):
        sl = slice(c * B_LOC, (c + 1) * B_LOC)
        in_maps.append(
            {
                "x2t": x2p[sl],
                "x3n": x3p[sl],
                # wl[p, b, ki, h] = wlc[b, ki*128+p, h]
                "wl": np.ascontiguousarray(
                    wlc[sl].reshape(B_LOC, KI, 128, H).transpose(2, 0, 1, 3)
                ),
                **shared,
            }
        )
    return in_maps, boe


_CACHE = {}


def kernel(**inputs) -> np.ndarray:
    from concourse.bass_utils import run_bass_kernel_spmd

    mode = os.environ.get("BASSK_MODE", "bf16")
    if mode not in _CACHE:
        _CACHE[mode] = build_program(mode)
    nc = _CACHE[mode]
    in_maps, boe = prep_inputs(inputs, mode)
    res = run_bass_kernel_spmd(nc, in_maps, list(range(N_CORES))).results
    out = np.concatenate([res[c]["out"] for c in range(N_CORES)], axis=0)
    return (out + boe[None, :]).astype(np.float32)


# revision 22
# speedup vs baseline: 1.0956x; 1.0956x over previous
"""Trainium2 Bass kernel: CrossAttention  (B=16, S=4096, D_IN=512, D=1024, H=16, HD=64).

reference math:
    x1e = x1@We1+be1; x2e = x2@We2+be2; x3e = x3@We2+be2
    q = x1e@Wq+bq; k = x2e@Wk+bk; v = x3e@Wv+bv     (per-head split, HD=64)
    attn = softmax(q.k/sqrt(HD)); av = attn.v; out = av@Wo+bo   -> [B, D]

Sharding: data-parallel over batch, 2 batches per core, 8 cores, no collectives.

Because the query is a SINGLE token per batch, q is tiny and is computed on
host; it is then folded into the K-side weights so the big K GEMM disappears:
    logits[h,s] = q'[h] . (x2[s]@W2k)[head h] = x2[s] @ wl[:,h],
        wl[:,h] = W2k[:, 64h:64h+64] @ q'[h],   q' = q/sqrt(HD)   (host, f64)
    (k bias shifts a head's logits by a constant -> softmax-invariant -> dropped)
The V GEMM is likewise eliminated by associativity:
    av = attn @ (x3@W2v) + bve = ((attn@x3) @ W2v) + bve
so the device only contracts attn (16 rows) against x3 once, then a tiny
[16,512]@[512,1024]. bve's contribution (attn rows sum to 1) is folded into
the output bias boe = bve@Wo + bo, added on host after the gather.

Device work per batch:
    lgts = wl^T @ x2^T  tiles        [H, S]     (x2 host-transposed, streamed)
    attn = exp(lgts - max), row sums kept; normalization folded into U
    attnT via PE transpose           [S, H]
    U    = attnT^T @ x3              [H, 512]   (x3 streamed in NATIVE layout)
    Un   = U * (1/sum)               [H, 512]
    O    = Un @ W2v                  [H, D];  av = diag-head blocks of O
    out  = blockdiag(av)^T @ Wo      [B_LOC, D]

Schedule: three DMA queues (gpsimd: x2, sync: x3, scalar: weights with wo
deferred to the tail), both batches' logits emitted back-to-back on the PE so
batch-1 streaming hides batch-0's softmax latency, per-tile partial maxes and
a split exp to shorten the softmax critical path.
"""

import os

import numpy as np

B, S, D_IN, D, H, HD = 16, 4096, 512, 1024, 16, 64
N_CORES = 8
B_LOC = B // N_CORES  # 2
KI = D_IN // 128      # 4 contraction chunks over D_IN
MO = D // 128         # 8 feature chunks over D
ST = S // 512         # 8 sequence tiles
SC = S // 128         # 32 sequence chunks
NT = D // 512         # 2 output-feature tiles


def _emit(nc, tc, ctx, mm_dt):
    import concourse.mybir as mybir

    dt = mybir.dt
    f32 = dt.float32
    AF = mybir.ActivationFunctionType
    AX = mybir.AxisListType
    ALU = mybir.AluOpType
    bf16 = mm_dt == dt.bfloat16
    t_dt = mm_dt if bf16 else f32

    # all activation/weight tensors host-packed tile-major: per-partition data
    # is contiguous (>=4KB lines) so every DMA runs at full HBM efficiency
    x2t = nc.declare_dram_parameter(
        "x2t", [B_LOC, ST, 128, KI, 512], mm_dt, isOutput=False
    )
    x3n = nc.declare_dram_parameter(
        "x3n", [B_LOC, ST, 128, 4, D_IN], mm_dt, isOutput=False
    )
    wl = nc.declare_dram_parameter("wl", [128, B_LOC, KI, H], mm_dt, isOutput=False)
    w2v = nc.declare_dram_parameter("w2v", [128, KI, D], mm_dt, isOutput=False)
    wo = nc.declare_dram_parameter("wo", [128, MO, D], mm_dt, isOutput=False)
    eye_io = nc.declare_dram_parameter("eye_io", [H, H], mm_dt, isOutput=False)
    eye_f32 = nc.declare_dram_parameter("eye_f32", [H, H], f32, isOutput=False)
    out_p = nc.declare_dram_parameter("out", [B_LOC, D], f32, isOutput=True)

    wpool = ctx.enter_context(tc.tile_pool(name="weights", bufs=1))
    x2pool = ctx.enter_context(tc.tile_pool(name="x2in", bufs=8))
    x3pool = ctx.enter_context(tc.tile_pool(name="x3in", bufs=16))
    bpool = ctx.enter_context(tc.tile_pool(name="perbatch", bufs=2))
    spool = ctx.enter_context(tc.tile_pool(name="singles", bufs=1))
    ps = ctx.enter_context(tc.tile_pool(name="ps", bufs=1, space="PSUM"))

    # --- small weights first (scalar queue), wo deferred to the tail ---
    eye_io_sb = spool.tile([H, H], mm_dt, tag="eye_io")
    nc.scalar.dma_start(out=eye_io_sb, in_=eye_io[:, :])
    eye_f32_sb = spool.tile([H, H], f32, tag="eye_f32")
    nc.scalar.dma_start(out=eye_f32_sb, in_=eye_f32[:, :])
    wl_sb = spool.tile([128, B_LOC, KI, H], mm_dt, tag="wl")
    nc.scalar.dma_start(out=wl_sb, in_=wl[:, :, :, :])
    w2v_sb = wpool.tile([128, KI, D], mm_dt, tag="w2v")
    nc.scalar.dma_start(out=w2v_sb, in_=w2v[:, :, :])

    # --- x2 on gpsimd, x3 on sync: both batches issued up front (a split
    # across more queues was tried and regressed: out-of-order tile arrival
    # stalls the in-order PE consumption) ---
    x2s_all = [[None] * ST for _ in range(B_LOC)]
    x3s_all = [[None] * ST for _ in range(B_LOC)]
    for b in range(B_LOC):
        for st in range(ST):
            x2s = x2pool.tile([128, KI, 512], mm_dt, tag="x2")
            nc.gpsimd.dma_start(out=x2s, in_=x2t[b, st])
            x2s_all[b][st] = x2s
    for b in range(B_LOC):
        for st in range(ST):
            x3s = x3pool.tile([128, 4, D_IN], mm_dt, tag="x3")
            nc.sync.dma_start(out=x3s, in_=x3n[b, st])
            x3s_all[b][st] = x3s

    lg_all, at_all, rs_all = [], [], []

    # ---------------- logits for BOTH batches back-to-back on PE ----------------
    for b in range(B_LOC):
        lg = bpool.tile([H, S], f32, tag="lg")
        nmx8 = bpool.tile([H, ST], f32, tag="nmx8")
        for st in range(ST):
            lp = ps.tile([H, 512], f32, tag="lg", bufs=2)
            for ki in range(KI):
                nc.tensor.matmul(
                    lp,
                    (wl_sb[:, b, ki, :]),
                    (x2s_all[b][st][:, ki, :]),
                    start=(ki == 0),
                    stop=(ki == KI - 1),
                )
            nc.vector.tensor_copy(out=lg[:, st * 512:(st + 1) * 512], in_=lp)
            # partial max per tile straight from PSUM (short final reduce)
            nc.vector.tensor_reduce(
                out=nmx8[:, st:st + 1], in_=lp, axis=AX.X, op=ALU.max
            )
        nmx = bpool.tile([H, 1], f32, tag="nmx")
        nc.vector.tensor_reduce(
            out=nmx, in_=nmx8, axis=AX.X, op=ALU.max, negate=True
        )

        # softmax: exp in two halves to release transposes earlier
        if bf16:
            attn = bpool.tile([H, S], mm_dt, tag="attn")
            attn_eye = eye_io_sb
        else:
            attn = lg  # exp in place
            attn_eye = eye_f32_sb
        ssum2 = bpool.tile([H, 2], f32, tag="ssum2")
        for hlf in range(2):
            sl = slice(hlf * (S // 2), (hlf + 1) * (S // 2))
            nc.scalar.activation(
                out=attn[:, sl],
                in_=lg[:, sl],
                func=AF.Exp,
                bias=nmx,
                scale=1.0,
                accum_out=ssum2[:, hlf:hlf + 1],
            )
        lg_all.append((attn, attn_eye, ssum2))

    # sums/reciprocals AFTER both batches' logits so batch-1 PSUM evictions on
    # the in-order vector queue never wait behind batch-0's exp
    for b in range(B_LOC):
        ssum2 = lg_all[b][2]
        ssum = bpool.tile([H, 1], f32, tag="ssum")
        nc.vector.tensor_scalar_add(
            out=ssum, in0=ssum2[:, 0:1], scalar1=ssum2[:, 1:2]
        )
        rs = bpool.tile([H, 1], f32, tag="rs")
        nc.vector.reciprocal(out=rs, in_=ssum)
        rs_all.append(rs)

    # attn^T via PE transpose AFTER both batches' logits (keeps PE streaming
    # batch 1 logits while batch 0's exp runs)
    for b in range(B_LOC):
        attn, attn_eye, _ = lg_all[b]
        at = bpool.tile([128, SC, H], mm_dt, tag="at")
        for g in range(SC // 4):
            tp = ps.tile([128, 4, H], t_dt, tag="tp", bufs=2)
            for j in range(4):
                sc = g * 4 + j
                nc.tensor.transpose(
                    tp[:, j, :], attn[:, sc * 128:(sc + 1) * 128], attn_eye
                )
            nc.vector.tensor_copy(out=at[:, g * 4:(g + 1) * 4, :], in_=tp)
        at_all.append(at)

    # ---------------- per-batch: U, O, diag extraction ----------------
    avv = spool.tile([128, MO, B_LOC], mm_dt, tag="avv")  # av^T diag blocks
    for b in range(B_LOC):
        at, rs = at_all[b], rs_all[b]
        up = ps.tile([H, D_IN], f32, tag="u", bufs=2, name=f"up{b}")
        for st in range(ST):
            for g in range(4):
                sc = st * 4 + g
                nc.tensor.matmul(
                    up,
                    (at[:, sc, :]),
                    (x3s_all[b][st][:, g, :]),
                    start=(sc == 0),
                    stop=(sc == SC - 1),
                )
        # normalize rows by 1/sum while evicting
        if bf16:
            un = bpool.tile([H, D_IN], mm_dt, tag="un")
            un_eye = eye_io_sb
        else:
            un = bpool.tile([H, D_IN], f32, tag="un")
            un_eye = eye_f32_sb
        nc.vector.tensor_scalar_mul(out=un, in0=up, scalar1=rs)

        # Un^T [D_IN, H] for the O GEMM
        unt = bpool.tile([128, KI, H], mm_dt, tag="unt")
        tpu = ps.tile([128, KI, H], t_dt, tag="tp", bufs=2)
        for c in range(KI):
            nc.tensor.transpose(
                tpu[:, c, :], un[:, c * 128:(c + 1) * 128], un_eye
            )
        nc.vector.tensor_copy(out=unt, in_=tpu)

        # O = Un @ W2v  [H, D]; av = diag-head blocks
        avs = bpool.tile([H, D], f32, tag="avs")
        for n in range(NT):
            op = ps.tile([H, 512], f32, tag="o", bufs=2)
            for ki in range(KI):
                nc.tensor.matmul(
                    op,
                    (unt[:, ki, :]),
                    (w2v_sb[:, ki, n * 512:(n + 1) * 512]),
                    start=(ki == 0),
                    stop=(ki == KI - 1),
                )
            nc.vector.tensor_copy(out=avs[:, n * 512:(n + 1) * 512], in_=op)

        # av^T, then extract the per-head diagonal blocks into avv
        avt = bpool.tile([128, MO, H], f32, tag="avt")
        for g in range(2):
            tpa = ps.tile([128, 4, H], f32, tag="tp", bufs=2)
            for j in range(4):
                mo = g * 4 + j
                nc.tensor.transpose(
                    tpa[:, j, :], avs[:, mo * 128:(mo + 1) * 128], eye_f32_sb
                )
            nc.vector.tensor_copy(out=avt[:, g * 4:(g + 1) * 4, :], in_=tpa)
        for mo in range(MO):
            nc.vector.tensor_copy(
                out=avv[0:64, mo, b:b + 1], in_=avt[0:64, mo, 2 * mo:2 * mo + 1]
            )
            nc.vector.tensor_copy(
                out=avv[64:128, mo, b:b + 1],
                in_=avt[64:128, mo, 2 * mo + 1:2 * mo + 2],
            )

    # wo arrives late on the scalar queue (emitted after both ACTIVATEs)
    wo_sb = wpool.tile([128, MO, D], mm_dt, tag="wo")
    nc.scalar.dma_start(out=wo_sb, in_=wo[:, :, :])

    # ---------------- out = avvec @ Wo  (both batches at once) ----------------
    out_sb = spool.tile([B_LOC, D], f32, tag="outsb")
    for n in range(NT):
        op = ps.tile([B_LOC, 512], f32, tag="lg", bufs=2)
        for mo in range(MO):
            nc.tensor.matmul(
                op,
                (avv[:, mo, :]),
                (wo_sb[:, mo, n * 512:(n + 1) * 512]),
                start=(mo == 0),
                stop=(mo == MO - 1),
            )
        nc.vector.tensor_copy(out=out_sb[:, n * 512:(n + 1) * 512], in_=op)
    nc.gpsimd.dma_start(out=out_p[:, :], in_=out_sb)


def build_program(mode=None):
    """mode: 'f32r' | 'bf16'. Returns a compiled Bass object."""
    from contextlib import ExitStack

    import concourse.mybir as mybir
    import concourse.tile as tile
    from concourse import bacc

    mode = mode or os.environ.get("BASSK_MODE", "bf16")
    mm_dt = {
        "f32": mybir.dt.float32,
        "f32r": mybir.dt.float32r,
        "bf16": mybir.dt.bfloat16,
    }[mode]

    nc = bacc.Bacc()
    with ExitStack() as ctx:
        tc = ctx.enter_context(tile.TileContext(nc))
        _emit(nc, tc, ctx, mm_dt)
    nc.compile()
    return nc


def prep_inputs(inputs, mode=None):
    """Host-side folding + per-core sharding. Returns (in_maps, boe)."""
    mode = mode or os.environ.get("BASSK_MODE", "bf16")
    g = {k: np.asarray(v, np.float64) for k, v in inputs.items()}
    W2k = g["We2"] @ g["Wk"]          # k bias dropped: softmax shift-invariant
    W2v = g["We2"] @ g["Wv"]
    q = (g["x1"][:, 0] @ g["We1"] + g["be1"]) @ g["Wq"] + g["bq"]   # [B, D]
    q /= np.sqrt(HD)
    # wl[b,:,h] = W2k[:, 64h:64h+64] @ q[b, 64h:64h+64]
    wl = np.einsum(
        "dhe,bhe->bdh", W2k.reshape(D_IN, H, HD), q.reshape(B, H, HD)
    )
    bve = g["be2"] @ g["Wv"] + g["bv"]
    boe = (bve @ g["Wo"] + g["bo"]).astype(np.float32)  # added on host at the end

    io_np = np.float32
    if mode == "bf16":
        import ml_dtypes

        io_np = ml_dtypes.bfloat16

    # tile-major packing: per-partition lines contiguous for full DMA rate
    x2 = np.asarray(inputs["x2"], np.float32).astype(io_np)
    x3 = np.asarray(inputs["x3"], np.float32).astype(io_np)
    # x2t[b, st, p, ki, s'] = x2[b, st*512+s', ki*128+p]
    x2p = np.ascontiguousarray(
        x2.reshape(B, ST, 512, KI, 128).transpose(0, 1, 4, 3, 2)
    )
    # x3n[b, st, p, g, d] = x3[b, (st*4+g)*128+p, d]
    x3p = np.ascontiguousarray(
        x3.reshape(B, ST, 4, 128, D_IN).transpose(0, 1, 3, 2, 4)
    )
    W2vp = np.ascontiguousarray(
        W2v.astype(np.float32).astype(io_np).reshape(KI, 128, D).transpose(1, 0, 2)
    )
    Wop = np.ascontiguousarray(
        np.asarray(inputs["Wo"], np.float32)
        .astype(io_np)
        .reshape(MO, 128, D)
        .transpose(1, 0, 2)
    )
    wlc = wl.astype(np.float32).astype(io_np)  # [B, D_IN, H]
    shared = {
        "w2v": W2vp,
        "wo": Wop,
        "eye_io": np.eye(H, dtype=io_np),
        "eye_f32": np.eye(H, dtype=np.float32),
    }
    in_maps = []
    for c in range(N_CORES):
        sl = slice(c * B_LOC, (c + 1) * B_LOC)
        in_maps.append(
            {
                "x2t": x2p[sl],
                "x3n": x3p[sl],
                # wl[p, b, ki, h] = wlc[b, ki*128+p, h]
                "wl": np.ascontiguousarray(
                    wlc[sl].reshape(B_LOC, KI, 128, H).transpose(2, 0, 1, 3)
                ),
                **shared,
            }
        )
    return in_maps, boe


_CACHE = {}


def kernel(**inputs) -> np.ndarray:
    from concourse.bass_utils import run_bass_kernel_spmd

    mode = os.environ.get("BASSK_MODE", "bf16")
    if mode not in _CACHE:
        _CACHE[mode] = build_program(mode)
    nc = _CACHE[mode]
    in_maps, boe = prep_inputs(inputs, mode)
    res = run_bass_kernel_spmd(nc, in_maps, list(range(N_CORES))).results
    out = np.concatenate([res[c]["out"] for c in range(N_CORES)], axis=0)
    return (out + boe[None, :]).astype(np.float32)
